# revision 18
# baseline (speedup 1.0000x reference)
"""Trainium2 Bass kernel for ConditionalLinearAttention.

Math (per batch element b, shapes hardcoded):
  xf  = x[b].reshape(256, 4096)
  cf  = cond_emb[b].reshape(512, 128)
  kv  = Wcond @ cf                      # (1024, 128)
  k   = softmax(kv[:512], per-row over the 128 cond positions)
  v   = kv[512:]
  ctx[h] = k_h @ v_h.T                  # (64, 64) per head h
  out = Wout @ apply(ctx) @ Wq @ xf + b_out

The whole attention folds into one per-batch matrix
W_comb = Wout @ ctx_blockdiag @ Wq (256x256); the spatial dimension then
sees ONE (256x256)@(256x4096) GEMM. Sharding: data-parallel over batch,
one batch element per core.

DMA strategy: the host pre-packs every input into the exact SBUF tile
image (one contiguous run per partition per dma_start), so each transfer
moves with ~4-13 KB descriptors instead of the ~2 KB shreds a strided
DRAM view produces. All streams are bf16 (tolerance 2e-2, measured error
~6e-3): input pack 1.625 MB + x 2 MB in, out 2 MB back.

Device dataflow per core (P=128 partitions). Softmax normalization is
folded into the context rows so exp stays un-normalized and no on-chip
transpose is needed:

  kvT (m,o)   = sum_j cf[:,j,:].T @ WcondT[:,j,:]      (4 MM, N=512 each half)
  expkT       = exp(kvT_k) in bf16                     (1 ACT op)
  Z cols      = expkT[:,128i:].T @ ones                (4 MM, N=1)
  ctx pair i  = expkT[:,128i:].T @ vT[:,128i:]         (4 MM, N=128)
                -> diagonal 64x64 blocks scaled by 1/Z into blockdiag ctx_bd
  A[:,i,:]    = ctx_bd[:,i,:].T @ Wq[:,i,:]            (4 MM, N=256)
  W_combT     = sum_kk A[:,kk,mc].T @ WoutT[:,kk,:]    (8 MM, N=256)
  OUT         = sum_ck W_combT[:,ck,mo].T @ x[:,ck,nt] + b  (32 MM, N=512)

Evictions of the phase-2 PSUM tiles alternate between the Scalar (ACT,
with the bias fused) and Vector engines so neither becomes the pole.
"""

import os

import numpy as np

B = 8
C = 256
N_SPATIAL = 4096  # 64*64
P = 128
N_CORES = 8

# pack column layout (bf16 elements): four interleaved [cf_j | wck_j]
# pieces (so the kv matmuls start as each piece lands), then wcv in two
# pieces, then wq, wo.
CW = 640                   # one [cf_j (128) | wck_j (512)] piece
WV0, WV1 = 2560, 4608      # wcondT v-half [p][j][o] 4x512
WQ0, WQ1 = 4608, 5632      # wq     [p][i][c]  4x256
WO0, WO1 = 5632, 6656      # woutT  [p][kk][o] 4x256
PACK_COLS = 6656

NQ = 4            # x/out chunks of 1024 spatial positions
QW = N_SPATIAL // NQ * 2  # 2048 cols per chunk tile (ck-major)

WARM = int(os.environ.get("KERNEL_WARM", "6"))  # PE warmup matmuls

_CACHE = {}
LAST_RESULTS = None  # BassKernelResults of the most recent run (for test.py)


def _build_nc():
    import concourse.bacc as bacc
    import concourse.mybir as mybir
    import concourse.tile as tile

    fp32 = mybir.dt.float32
    bf16 = mybir.dt.bfloat16
    AF = mybir.ActivationFunctionType

    nc = bacc.Bacc("TRN2", target_bir_lowering=False, debug=False,
                   num_devices=N_CORES)

    pack_t = nc.dram_tensor("pack", [P, PACK_COLS], bf16, kind="ExternalInput").ap()
    x_t = nc.dram_tensor("x", [P, 2 * N_SPATIAL], bf16, kind="ExternalInput").ap()
    bias_t = nc.dram_tensor("bias", [P, 2], fp32, kind="ExternalInput").ap()
    out_t = nc.dram_tensor("out", [P, 2 * N_SPATIAL], bf16, kind="ExternalOutput").ap()

    with tile.TileContext(nc) as tc:
        with (
            tc.tile_pool(name="main", bufs=1) as mainp,
            tc.tile_pool(name="work", bufs=2) as workp,
            tc.tile_pool(name="outp", bufs=4) as outp,
            tc.tile_pool(name="ps", bufs=4, space="PSUM") as psp,
            tc.tile_pool(name="psO", bufs=4, space="PSUM") as psO,
        ):
            # warmup operand tiles first: junk matmuls must be runnable the
            # moment the engines clear the entry rendezvous
            wl = mainp.tile([P, P], bf16)
            nc.gpsimd.memset(wl, 0.0)
            wz = mainp.tile([P, 512], bf16)
            nc.gpsimd.memset(wz, 0.0)

            # --- input DMAs, critical-path order on the sync (SP) HWDGE ring
            wA = mainp.tile([P, 4 * CW], bf16)      # 4x [cf_j | wck_j]
            for j in range(4):
                nc.sync.dma_start(wA[:, CW * j:CW * (j + 1)],
                                  pack_t[:, CW * j:CW * (j + 1)])
            wV = mainp.tile([P, WV1 - WV0], bf16)   # wcv, two pieces
            nc.sync.dma_start(wV[:, 0:1024], pack_t[:, WV0:WV0 + 1024])
            nc.sync.dma_start(wV[:, 1024:2048], pack_t[:, WV0 + 1024:WV1])
            wQ = mainp.tile([P, WQ1 - WQ0], bf16)   # wq
            nc.sync.dma_start(wQ, pack_t[:, WQ0:WQ1])
            wO = mainp.tile([P, WO1 - WO0], bf16)   # wo
            nc.sync.dma_start(wO, pack_t[:, WO0:WO1])
            x_sb = []
            for q in range(NQ):
                t = mainp.tile([P, QW], bf16, tag=f"x{q}")
                nc.sync.dma_start(t, x_t[:, QW * q:QW * (q + 1)])
                x_sb.append(t)
            # bias: tiny -> keep off the SP ring
            bias_sb = mainp.tile([P, 2], fp32)
            nc.gpsimd.dma_start(bias_sb, bias_t)
            ones_sb = mainp.tile([P, 1], bf16)
            nc.vector.memset(ones_sb, 1.0)
            # block-diagonal ctx staging: zero the off-diagonal blocks early,
            # off the critical path (gpsimd has nothing else to do)
            ctx_bd = mainp.tile([P, 4, 128], bf16)
            nc.gpsimd.memset(ctx_bd, 0.0)

            # PE warmup: junk matmuls with no DMA deps fill the otherwise-idle
            # input-DMA window so HAM unthrottles (1.2 -> 2.4 GHz) before the
            # real matmuls start
            def keep_warm(n):
                for _ in range(n):
                    pj = psO.tile([P, 512], fp32, tag="O")
                    nc.tensor.matmul(pj, wl, wz, start=True, stop=True)

            keep_warm(WARM)

            # --- phase 1: per-batch W_comb (256x256) ---
            # kvT (cond position m on partitions): k half, then v half
            pkv = psp.tile([P, 512], fp32, tag="p1")
            for j in range(4):
                nc.tensor.matmul(pkv, wA[:, CW * j:CW * j + 128],
                                 wA[:, CW * j + 128:CW * (j + 1)],
                                 start=(j == 0), stop=(j == 3))
            # exp in two column halves so the Z/ctx chains start earlier
            expkT = mainp.tile([P, 512], bf16)
            nc.scalar.activation(out=expkT[:, 0:256], in_=pkv[:, 0:256],
                                 func=AF.Exp)
            nc.scalar.activation(out=expkT[:, 256:512], in_=pkv[:, 256:512],
                                 func=AF.Exp)

            pvv = psp.tile([P, 512], fp32, tag="p1")
            for j in range(4):
                nc.tensor.matmul(pvv, wA[:, CW * j:CW * j + 128],
                                 wV[:, 512 * j:512 * (j + 1)],
                                 start=(j == 0), stop=(j == 3))
            # v rows to SBUF: first half on the vector engine, second half on
            # the scalar engine (free after exp), so ctx pairs unblock sooner
            vT = mainp.tile([P, 512], bf16)
            nc.vector.tensor_copy(out=vT[:, 0:128], in_=pvv[:, 0:128])
            nc.vector.tensor_copy(out=vT[:, 128:256], in_=pvv[:, 128:256])
            nc.scalar.activation(out=vT[:, 256:384], in_=pvv[:, 256:384],
                                 func=AF.Identity)
            nc.scalar.activation(out=vT[:, 384:512], in_=pvv[:, 384:512],
                                 func=AF.Identity)

            # softmax denominators as columns: Z[hd] = sum_m expkT[m, hd].
            # All four pair-columns land in ONE psum tile -> one reciprocal.
            pz = psp.tile([P, 4], fp32, tag="p1")
            for i in range(4):
                nc.tensor.matmul(pz[:, i:i + 1], expkT[:, 128 * i:128 * (i + 1)],
                                 ones_sb, start=True, stop=True)
            rcols = workp.tile([P, 4], fp32)
            nc.vector.reciprocal(rcols, pz)
            rcol = [rcols[:, i:i + 1] for i in range(4)]
            keep_warm(1)

            # per-head-pair context; scale rows by 1/Z while extracting the
            # diagonal 64x64 blocks into the block-diagonal layout
            # (ctx_bd allocated + zeroed above, before the DMAs).
            for i in range(4):
                pc = psp.tile([P, 128], fp32, tag="p1")
                nc.tensor.matmul(pc, expkT[:, 128 * i:128 * (i + 1)],
                                 vT[:, 128 * i:128 * (i + 1)], start=True, stop=True)
                if i < 2:
                    nc.vector.tensor_scalar_mul(ctx_bd[0:64, i, 0:64],
                                                pc[0:64, 0:64], rcol[i][0:64])
                    nc.vector.tensor_scalar_mul(ctx_bd[64:128, i, 64:128],
                                                pc[64:128, 64:128], rcol[i][64:128])
                else:
                    nc.scalar.activation(out=ctx_bd[0:64, i, 0:64],
                                         in_=pc[0:64, 0:64], func=AF.Identity,
                                         scale=rcol[i][0:64])
                    nc.scalar.activation(out=ctx_bd[64:128, i, 64:128],
                                         in_=pc[64:128, 64:128], func=AF.Identity,
                                         scale=rcol[i][64:128])
                if i in (1, 3):
                    keep_warm(1)

            # A[he, c] = blockdiag(ctx).T @ Wq  (k-tile i = head pair i)
            A_sb = mainp.tile([P, 4, 256], bf16)
            for i in range(4):
                pa = psp.tile([P, 256], fp32, tag="p1")
                nc.tensor.matmul(pa, ctx_bd[:, i, :],
                                 wQ[:, 256 * i:256 * (i + 1)],
                                 start=True, stop=True)
                if i % 2 == 0:
                    nc.vector.tensor_copy(out=A_sb[:, i, :], in_=pa)
                else:
                    nc.scalar.activation(out=A_sb[:, i, :], in_=pa,
                                         func=AF.Identity)
                if i == 1:
                    keep_warm(1)

            # W_combT[c, o] = sum_he A[he, c] * WoutT[he, o]. Interleave the
            # two mc accumulation groups so both finish (and evict on separate
            # engines) as soon as the last A chunk lands.
            wc_sb = mainp.tile([P, 2, 256], bf16)
            pw0 = psp.tile([P, 256], fp32, tag="p1")
            pw1 = psp.tile([P, 256], fp32, tag="p1")
            pws = [pw0, pw1]
            for kk in range(4):
                for mc in range(2):
                    nc.tensor.matmul(pws[mc], A_sb[:, kk, 128 * mc:128 * (mc + 1)],
                                     wO[:, 256 * kk:256 * (kk + 1)],
                                     start=(kk == 0), stop=(kk == 3))
            nc.vector.tensor_copy(out=wc_sb[:, 0, :], in_=pws[0])
            nc.scalar.activation(out=wc_sb[:, 1, :], in_=pws[1],
                                 func=AF.Identity)

            # --- phase 2: OUT = W_comb @ xf + bias, streamed over x chunks
            for q in range(NQ):
                ot = outp.tile([P, QW], bf16, tag="osb")
                for mo in range(2):
                    for sub in range(2):
                        po = psO.tile([P, 512], fp32, tag="O")
                        for ck in range(2):
                            nc.tensor.matmul(
                                po, wc_sb[:, ck, 128 * mo:128 * (mo + 1)],
                                x_sb[q][:, 1024 * ck + 512 * sub:
                                        1024 * ck + 512 * (sub + 1)],
                                start=(ck == 0), stop=(ck == 1))
                        dst = ot[:, 1024 * mo + 512 * sub:
                                 1024 * mo + 512 * (sub + 1)]
                        if sub == 0:
                            nc.scalar.activation(out=dst, in_=po,
                                                 func=AF.Identity,
                                                 bias=bias_sb[:, mo:mo + 1],
                                                 scale=1.0)
                        else:
                            nc.vector.tensor_scalar_add(out=dst, in0=po,
                                                        scalar1=bias_sb[:, mo:mo + 1])
                eng = nc.scalar if q % 2 == 0 else nc.sync
                eng.dma_start(out_t[:, QW * q:QW * (q + 1)], ot)

    nc.compile()
    return nc


def _pack_host(x, cond_emb, Wq, Wcond, Wout, b_out):
    import ml_dtypes

    bf = ml_dtypes.bfloat16
    xf = x.reshape(B, 2, 128, NQ, 1024)
    x_host = np.ascontiguousarray(
        xf.transpose(0, 2, 3, 1, 4).reshape(B, P, 2 * N_SPATIAL)).astype(bf)

    WcondT = Wcond.T  # (512, 1024)
    # [p][j][o] views of the k/v weight halves
    wck = WcondT[:, :512].reshape(4, 128, 512).transpose(1, 0, 2)   # (128,4,512)
    wcv = WcondT[:, 512:].reshape(4, 128, 512).transpose(1, 0, 2)
    wqp = Wq.reshape(4, 128, 256).transpose(1, 0, 2).reshape(P, 1024)
    wop = Wout.T.reshape(4, 128, 256).transpose(1, 0, 2).reshape(P, 1024)
    # [p][j][m] view of cf, per batch
    cf = cond_emb.reshape(B, 4, 128, 128).transpose(0, 2, 1, 3)     # (B,128,4,128)

    packs = []
    for b in range(B):
        pieces = []
        for j in range(4):  # interleaved [cf_j | wck_j]
            pieces.append(cf[b, :, j, :])
            pieces.append(wck[:, j, :])
        pieces.append(wcv.reshape(P, 2048))
        pieces.append(wqp)
        pieces.append(wop)
        packs.append(np.ascontiguousarray(
            np.concatenate(pieces, axis=1).astype(bf)))

    bias_host = np.ascontiguousarray(
        b_out.reshape(2, 128).T).astype(np.float32)  # (128, 2)
    return x_host, packs, bias_host


def kernel(x, cond_emb, Wq, Wcond, Wout, b_out):
    from concourse.bass_utils import run_bass_kernel_spmd

    global LAST_RESULTS

    if "nc" not in _CACHE:
        _CACHE["nc"] = _build_nc()
    nc = _CACHE["nc"]

    x_host, packs, bias_host = _pack_host(
        np.asarray(x), np.asarray(cond_emb), np.asarray(Wq),
        np.asarray(Wcond), np.asarray(Wout), np.asarray(b_out))

    in_maps = [
        {
            "pack": packs[b],
            "x": np.ascontiguousarray(x_host[b]),
            "bias": bias_host,
        }
        for b in range(B)
    ]

    trace = bool(int(os.environ.get("KERNEL_TRACE", "0")))
    res = run_bass_kernel_spmd(nc, in_maps, core_ids=list(range(N_CORES)),
                               trace=trace)
    LAST_RESULTS = res
    out = np.stack([
        np.asarray(res.results[b]["out"])
        .reshape(P, NQ, 2, 1024).transpose(2, 0, 1, 3).reshape(C, N_SPATIAL)
        for b in range(B)
    ])
    return out.reshape(B, C, 64, 64).astype(np.float32)


if __name__ == "__main__":
    xs = np.random.RandomState(0)
    ins = {
        "x": xs.randn(8, 256, 64, 64).astype(np.float32),
        "cond_emb": xs.randn(8, 512, 1, 128).astype(np.float32),
        "Wq": (xs.randn(512, 256) * 0.05).astype(np.float32),
        "Wcond": (xs.randn(1024, 512) * 0.05).astype(np.float32),
        "Wout": (xs.randn(256, 512) * 0.05).astype(np.float32),
        "b_out": np.zeros(256, np.float32),
    }
    o = kernel(**ins)
    print("ran, shape", o.shape)


# revision 28
# speedup vs baseline: 1.1262x; 1.1262x over previous
"""Trainium2 Bass kernel for ConditionalLinearAttention.

Math (per batch element b, shapes hardcoded):
  xf  = x[b].reshape(256, 4096)
  cf  = cond_emb[b].reshape(512, 128)
  kv  = Wcond @ cf                      # (1024, 128)
  k   = softmax(kv[:512], per-row over the 128 cond positions)
  v   = kv[512:]
  ctx[h] = k_h @ v_h.T                  # (64, 64) per head h
  out = Wout @ apply(ctx) @ Wq @ xf + b_out

The whole attention folds into one per-batch matrix
W_comb = Wout @ ctx_blockdiag @ Wq (256x256); the spatial dimension then
sees ONE (256x256)@(256x4096) GEMM. Sharding: data-parallel over batch,
one batch element per core.

DMA strategy: the host pre-packs every input into the exact SBUF tile
image (one contiguous run per partition per dma_start), so each transfer
moves with ~4-13 KB descriptors instead of the ~2 KB shreds a strided
DRAM view produces. All streams are bf16 (tolerance 2e-2, measured error
~6e-3): input pack 1.625 MB + x 2 MB in, out 2 MB back.

Device dataflow per core (P=128 partitions). Softmax normalization is
folded into the context rows so exp stays un-normalized and no on-chip
transpose is needed:

  kvT (m,o)   = sum_j cf[:,j,:].T @ WcondT[:,j,:]      (4 MM, N=512 each half)
  expkT       = exp(kvT_k) in bf16                     (1 ACT op)
  Z cols      = expkT[:,128i:].T @ ones                (4 MM, N=1)
  ctx pair i  = expkT[:,128i:].T @ vT[:,128i:]         (4 MM, N=128)
                -> diagonal 64x64 blocks scaled by 1/Z into blockdiag ctx_bd
  A[:,i,:]    = ctx_bd[:,i,:].T @ Wq[:,i,:]            (4 MM, N=256)
  W_combT     = sum_kk A[:,kk,mc].T @ WoutT[:,kk,:]    (8 MM, N=256)
  OUT         = sum_ck W_combT[:,ck,mo].T @ x[:,ck,nt] + b  (32 MM, N=512)

Evictions of the phase-2 PSUM tiles alternate between the Scalar (ACT,
with the bias fused) and Vector engines so neither becomes the pole.
"""

import os

import numpy as np

B = 8
C = 256
N_SPATIAL = 4096  # 64*64
P = 128
N_CORES = 8

# pack column layout (bf16 elements): four interleaved [cf_j | wck_j]
# pieces (so the kv matmuls start as each piece lands), then wcv in two
# pieces, then wq, wo.
CW = 640                   # one [cf_j (128) | wck_j (512)] piece
WV0, WV1 = 2560, 4608      # wcondT v-half [p][j][o] 4x512
WQ0, WQ1 = 4608, 5632      # wq     [p][i][c]  4x256
WO0, WO1 = 5632, 6656      # woutT  [p][kk][o] 4x256
PACK_COLS = 6656

NQ = 4            # x/out chunks of 1024 spatial positions
QW = N_SPATIAL // NQ * 2  # 2048 cols per chunk tile (ck-major)

WARM = int(os.environ.get("KERNEL_WARM", "6"))  # PE warmup matmuls

_CACHE = {}
LAST_RESULTS = None  # BassKernelResults of the most recent run (for test.py)


def _build_nc():
    import concourse.bacc as bacc
    import concourse.mybir as mybir
    import concourse.tile as tile

    fp32 = mybir.dt.float32
    bf16 = mybir.dt.bfloat16
    AF = mybir.ActivationFunctionType

    nc = bacc.Bacc("TRN2", target_bir_lowering=False, debug=False,
                   num_devices=N_CORES)

    pack_t = nc.dram_tensor("pack", [P, PACK_COLS], bf16, kind="ExternalInput").ap()
    x_t = nc.dram_tensor("x", [P, 2 * N_SPATIAL], bf16, kind="ExternalInput").ap()
    bias_t = nc.dram_tensor("bias", [P, 2], fp32, kind="ExternalInput").ap()
    out_t = nc.dram_tensor("out", [P, 2 * N_SPATIAL], bf16, kind="ExternalOutput").ap()

    with tile.TileContext(nc) as tc:
        with (
            tc.tile_pool(name="main", bufs=1) as mainp,
            tc.tile_pool(name="work", bufs=2) as workp,
            tc.tile_pool(name="outp", bufs=4) as outp,
            tc.tile_pool(name="ps", bufs=4, space="PSUM") as psp,
            tc.tile_pool(name="psO", bufs=4, space="PSUM") as psO,
        ):
            # warmup operand tiles first: junk matmuls must be runnable the
            # moment the engines clear the entry rendezvous
            wl = mainp.tile([P, P], bf16)
            nc.gpsimd.memset(wl, 0.0)
            wz = mainp.tile([P, 512], bf16)
            nc.gpsimd.memset(wz, 0.0)

            # --- input DMAs, critical-path order on the sync (SP) HWDGE ring
            wA = mainp.tile([P, 4 * CW], bf16)      # 4x [cf_j | wck_j]
            nc.sync.dma_start(wA, pack_t[:, 0:4 * CW])
            wV = mainp.tile([P, WV1 - WV0], bf16)   # wcv
            nc.sync.dma_start(wV, pack_t[:, WV0:WV1])
            wC = mainp.tile([P, WO1 - WQ0], bf16)   # wq + wo
            nc.sync.dma_start(wC, pack_t[:, WQ0:WO1])
            wQ = wC[:, 0:1024]
            wO = wC[:, 1024:2048]
            # x in three pieces: early q0 (phase-2 can start), middle q1+q2,
            # small q3 so the output tail pipelines tightly
            x0 = mainp.tile([P, QW], bf16)
            nc.sync.dma_start(x0, x_t[:, 0:QW])
            x12 = mainp.tile([P, 2 * QW], bf16)
            nc.sync.dma_start(x12, x_t[:, QW:3 * QW])
            x3 = mainp.tile([P, QW], bf16)
            nc.sync.dma_start(x3, x_t[:, 3 * QW:4 * QW])
            x_sb = [x0, x12[:, 0:QW], x12[:, QW:2 * QW], x3]
            # bias: tiny -> keep off the SP ring
            bias_sb = mainp.tile([P, 2], fp32)
            nc.gpsimd.dma_start(bias_sb, bias_t)
            ones_sb = mainp.tile([P, 1], bf16)
            nc.vector.memset(ones_sb, 1.0)
            # block-diagonal ctx staging: zero the off-diagonal blocks early,
            # off the critical path (gpsimd has nothing else to do)
            ctx_bd = mainp.tile([P, 4, 128], bf16)
            nc.gpsimd.memset(ctx_bd, 0.0)

            # PE warmup: junk matmuls with no DMA deps fill the otherwise-idle
            # input-DMA window so HAM unthrottles (1.2 -> 2.4 GHz) before the
            # real matmuls start. Narrow (N=128) sprinkles keep the PE stream
            # dense through phase-1 dependency gaps without delaying real
            # matmuls by more than ~100ns each.
            def keep_warm(n):
                for _ in range(n):
                    pj = psO.tile([P, 512], fp32, tag="O")
                    nc.tensor.matmul(pj, wl, wz, start=True, stop=True)

            def warm64(n):
                for _ in range(n):
                    pj = psO.tile([P, 512], fp32, tag="O")
                    nc.tensor.matmul(pj[0:64, 0:64], wl[:, 0:64], wl[:, 0:64],
                                     start=True, stop=True)

            keep_warm(WARM)

            # --- phase 1: per-batch W_comb (256x256) ---
            # kvT (cond position m on partitions): k half, then v half
            pkv = psp.tile([P, 512], fp32, tag="p1")
            for j in range(4):
                nc.tensor.matmul(pkv, wA[:, CW * j:CW * j + 128],
                                 wA[:, CW * j + 128:CW * (j + 1)],
                                 start=(j == 0), stop=(j == 3))
            warm64(3)
            # exp in two column halves so the Z/ctx chains start earlier
            expkT = mainp.tile([P, 512], bf16)
            nc.scalar.activation(out=expkT[:, 0:256], in_=pkv[:, 0:256],
                                 func=AF.Exp)
            nc.scalar.activation(out=expkT[:, 256:512], in_=pkv[:, 256:512],
                                 func=AF.Exp)

            pvv = psp.tile([P, 512], fp32, tag="p1")
            for j in range(4):
                nc.tensor.matmul(pvv, wA[:, CW * j:CW * j + 128],
                                 wV[:, 512 * j:512 * (j + 1)],
                                 start=(j == 0), stop=(j == 3))
            warm64(4)
            # v rows to SBUF: first half on the vector engine, second half on
            # the scalar engine (free after exp), so ctx pairs unblock sooner
            vT = mainp.tile([P, 512], bf16)
            nc.vector.tensor_copy(out=vT[:, 0:128], in_=pvv[:, 0:128])
            nc.vector.tensor_copy(out=vT[:, 128:256], in_=pvv[:, 128:256])
            nc.scalar.activation(out=vT[:, 256:384], in_=pvv[:, 256:384],
                                 func=AF.Identity)
            nc.scalar.activation(out=vT[:, 384:512], in_=pvv[:, 384:512],
                                 func=AF.Identity)

            # softmax denominators as columns: Z[hd] = sum_m expkT[m, hd].
            # All four pair-columns land in ONE psum tile -> one reciprocal.
            pz = psp.tile([P, 4], fp32, tag="p1")
            for i in range(4):
                nc.tensor.matmul(pz[:, i:i + 1], expkT[:, 128 * i:128 * (i + 1)],
                                 ones_sb, start=True, stop=True)
            rcols = workp.tile([P, 4], fp32)
            nc.vector.reciprocal(rcols, pz)
            rcol = [rcols[:, i:i + 1] for i in range(4)]
            warm64(4)

            # per-head-pair context; scale rows by 1/Z while extracting the
            # diagonal 64x64 blocks into the block-diagonal layout
            # (ctx_bd allocated + zeroed above, before the DMAs).
            for i in range(4):
                pc = psp.tile([P, 128], fp32, tag="p1")
                nc.tensor.matmul(pc, expkT[:, 128 * i:128 * (i + 1)],
                                 vT[:, 128 * i:128 * (i + 1)], start=True, stop=True)
                if i < 2:
                    nc.vector.tensor_scalar_mul(ctx_bd[0:64, i, 0:64],
                                                pc[0:64, 0:64], rcol[i][0:64])
                    nc.vector.tensor_scalar_mul(ctx_bd[64:128, i, 64:128],
                                                pc[64:128, 64:128], rcol[i][64:128])
                else:
                    nc.scalar.activation(out=ctx_bd[0:64, i, 0:64],
                                         in_=pc[0:64, 0:64], func=AF.Identity,
                                         scale=rcol[i][0:64])
                    nc.scalar.activation(out=ctx_bd[64:128, i, 64:128],
                                         in_=pc[64:128, 64:128], func=AF.Identity,
                                         scale=rcol[i][64:128])
                warm64(3)

            # A[he, c] = blockdiag(ctx).T @ Wq  (k-tile i = head pair i)
            A_sb = mainp.tile([P, 4, 256], bf16)
            for i in range(4):
                pa = psp.tile([P, 256], fp32, tag="p1")
                nc.tensor.matmul(pa, ctx_bd[:, i, :],
                                 wQ[:, 256 * i:256 * (i + 1)],
                                 start=True, stop=True)
                if i % 2 == 0:
                    nc.vector.tensor_copy(out=A_sb[:, i, :], in_=pa)
                else:
                    nc.scalar.activation(out=A_sb[:, i, :], in_=pa,
                                         func=AF.Identity)
                warm64(2)

            # W_combT[c, o] = sum_he A[he, c] * WoutT[he, o]. Interleave the
            # two mc accumulation groups so both finish (and evict on separate
            # engines) as soon as the last A chunk lands.
            wc_sb = mainp.tile([P, 2, 256], bf16)
            pw0 = psp.tile([P, 256], fp32, tag="p1")
            pw1 = psp.tile([P, 256], fp32, tag="p1")
            pws = [pw0, pw1]
            for kk in range(4):
                for mc in range(2):
                    nc.tensor.matmul(pws[mc], A_sb[:, kk, 128 * mc:128 * (mc + 1)],
                                     wO[:, 256 * kk:256 * (kk + 1)],
                                     start=(kk == 0), stop=(kk == 3))
                if kk < 3:
                    warm64(1)
            nc.vector.tensor_copy(out=wc_sb[:, 0, :], in_=pws[0])
            nc.scalar.activation(out=wc_sb[:, 1, :], in_=pws[1],
                                 func=AF.Identity)

            # --- phase 2: OUT = W_comb @ xf + bias, streamed over x chunks
            for q in range(NQ):
                ot = outp.tile([P, QW], bf16, tag="osb")
                for mo in range(2):
                    for sub in range(2):
                        po = psO.tile([P, 512], fp32, tag="O")
                        for ck in range(2):
                            nc.tensor.matmul(
                                po, wc_sb[:, ck, 128 * mo:128 * (mo + 1)],
                                x_sb[q][:, 1024 * ck + 512 * sub:
                                        1024 * ck + 512 * (sub + 1)],
                                start=(ck == 0), stop=(ck == 1))
                        dst = ot[:, 1024 * mo + 512 * sub:
                                 1024 * mo + 512 * (sub + 1)]
                        if sub == 0:
                            nc.scalar.activation(out=dst, in_=po,
                                                 func=AF.Identity,
                                                 bias=bias_sb[:, mo:mo + 1],
                                                 scale=1.0)
                        else:
                            nc.vector.tensor_scalar_add(out=dst, in0=po,
                                                        scalar1=bias_sb[:, mo:mo + 1])
                eng = nc.scalar if q % 2 == 0 else nc.sync
                eng.dma_start(out_t[:, QW * q:QW * (q + 1)], ot)

    nc.compile()
    return nc


def _pack_host(x, cond_emb, Wq, Wcond, Wout, b_out):
    import ml_dtypes

    bf = ml_dtypes.bfloat16
    xf = x.reshape(B, 2, 128, NQ, 1024)
    x_host = np.ascontiguousarray(
        xf.transpose(0, 2, 3, 1, 4).reshape(B, P, 2 * N_SPATIAL)).astype(bf)

    WcondT = Wcond.T  # (512, 1024)
    # [p][j][o] views of the k/v weight halves
    wck = WcondT[:, :512].reshape(4, 128, 512).transpose(1, 0, 2)   # (128,4,512)
    wcv = WcondT[:, 512:].reshape(4, 128, 512).transpose(1, 0, 2)
    wqp = Wq.reshape(4, 128, 256).transpose(1, 0, 2).reshape(P, 1024)
    wop = Wout.T.reshape(4, 128, 256).transpose(1, 0, 2).reshape(P, 1024)
    # [p][j][m] view of cf, per batch
    cf = cond_emb.reshape(B, 4, 128, 128).transpose(0, 2, 1, 3)     # (B,128,4,128)

    packs = []
    for b in range(B):
        pieces = []
        for j in range(4):  # interleaved [cf_j | wck_j]
            pieces.append(cf[b, :, j, :])
            pieces.append(wck[:, j, :])
        pieces.append(wcv.reshape(P, 2048))
        pieces.append(wqp)
        pieces.append(wop)
        packs.append(np.ascontiguousarray(
            np.concatenate(pieces, axis=1).astype(bf)))

    bias_host = np.ascontiguousarray(
        b_out.reshape(2, 128).T).astype(np.float32)  # (128, 2)
    return x_host, packs, bias_host


def kernel(x, cond_emb, Wq, Wcond, Wout, b_out):
    from concourse.bass_utils import run_bass_kernel_spmd

    global LAST_RESULTS

    if "nc" not in _CACHE:
        _CACHE["nc"] = _build_nc()
    nc = _CACHE["nc"]

    x_host, packs, bias_host = _pack_host(
        np.asarray(x), np.asarray(cond_emb), np.asarray(Wq),
        np.asarray(Wcond), np.asarray(Wout), np.asarray(b_out))

    in_maps = [
        {
            "pack": packs[b],
            "x": np.ascontiguousarray(x_host[b]),
            "bias": bias_host,
        }
        for b in range(B)
    ]

    trace = bool(int(os.environ.get("KERNEL_TRACE", "0")))
    res = run_bass_kernel_spmd(nc, in_maps, core_ids=list(range(N_CORES)),
                               trace=trace)
    LAST_RESULTS = res
    out = np.stack([
        np.asarray(res.results[b]["out"])
        .reshape(P, NQ, 2, 1024).transpose(2, 0, 1, 3).reshape(C, N_SPATIAL)
        for b in range(B)
    ])
    return out.reshape(B, C, 64, 64).astype(np.float32)


if __name__ == "__main__":
    xs = np.random.RandomState(0)
    ins = {
        "x": xs.randn(8, 256, 64, 64).astype(np.float32),
        "cond_emb": xs.randn(8, 512, 1, 128).astype(np.float32),
        "Wq": (xs.randn(512, 256) * 0.05).astype(np.float32),
        "Wcond": (xs.randn(1024, 512) * 0.05).astype(np.float32),
        "Wout": (xs.randn(256, 512) * 0.05).astype(np.float32),
        "b_out": np.zeros(256, np.float32),
    }
    o = kernel(**ins)
    print("ran, shape", o.shape)
